# revision 1
# baseline (speedup 1.0000x reference)
"""Trainium2 Bass kernel for nn_MultiHeadLinearAttention.

Sharding: batch (4-way) x head-half (2-way) = 8 cores, no cross-core
reductions (the kv / summed_k contractions are over L, which stays local).

Per core (b = core//2, hh = core%2, 8 heads, F=512 local feature cols):
  phase A (per 512-token block):
    Q^T = fm(Wq_h @ X_q^T + bq)          (feat-major, f32r matmuls, N=512)
    K   = fm(X_k Wk_h^T + bk)            (token-major; bias via rank-1 matmul)
    V1  = [(X_v Wv_h^T + bv) * m_kv^2/L | m_kv]
    kv[h] += K_h^T @ V1_h                (fp32, accumulated in PSUM all L)
  phase B (per 128-token tile): out = (Q^T)^T @ blockdiag(kv) scaled by
    m_q*L / (eps + m_q * (q . summed_k)), summed_k taken from the mask
    column of kv.
fm(y) = elu(y)+1 = min(exp(y),1) + relu(y).
"""

import os
import sys
import types

for _p in ("/opt/trn_rl_repo",):
    if _p not in sys.path:
        sys.path.insert(0, _p)
os.environ.setdefault("MYCRO_LOCAL_CACHE", "1")

import numpy as np
import ml_dtypes

# Optional NTFF profiling support (used by test harness via TRN_KERNEL_TRACE=1).
if "antenv.axon_hooks" not in sys.modules:
    _hook_mod = types.ModuleType("antenv.axon_hooks")
    _hook_state = {"h": None}
    _hook_mod.set_axon_ntff_profile_hook = lambda h: _hook_state.__setitem__("h", h)
    _hook_mod.get_axon_ntff_profile_hook = lambda: _hook_state["h"]
    sys.modules["antenv.axon_hooks"] = _hook_mod
    try:
        from trn_agent_boot.trn_boot import _ntff_profile_via_ctypes
        _hook_state["h"] = _ntff_profile_via_ctypes("/opt/axon/libaxon_pjrt.so")
    except Exception:
        pass

import concourse.bacc as bacc
import concourse.tile as tile
import concourse.mybir as mybir
import concourse.bass_utils as bass_utils
from concourse.bass import ts, ds
from concourse.bass_utils import run_bass_kernel_spmd

bass_utils.upload_artifacts = lambda tmpdir: tmpdir  # no artifact bucket here

F32 = mybir.dt.float32
F32R = mybir.dt.float32r
BF16 = mybir.dt.bfloat16
ACTF = mybir.ActivationFunctionType
ALU = mybir.AluOpType

B, L, C, H, D = 4, 4096, 1024, 16, 64
EPS = 1e-6
NCORES = 8
HPC = H // NCORES * 4          # 8 heads per core
F = HPC * D                    # 512 local feature cols
KC = C // 128                  # 8 contraction chunks
NBLK = L // 512                # 8 token blocks (phase A)
NST = L // 128                 # 32 token subtiles
FT = F // 128                  # 4 feature tiles (head pairs)

_PROG = None


def _build_program():
    nc = bacc.Bacc("TRN2", target_bir_lowering=False, debug=False,
                   num_devices=NCORES)

    xtq_d = nc.dram_tensor("xtq", [C, L], F32R, kind="ExternalInput")
    xtk_d = nc.dram_tensor("xtk", [C, L], F32R, kind="ExternalInput")
    xtv_d = nc.dram_tensor("xtv", [C, L], F32R, kind="ExternalInput")
    wtq_d = nc.dram_tensor("wtq", [C, F], F32R, kind="ExternalInput")
    wtk_d = nc.dram_tensor("wtk", [C, F], F32R, kind="ExternalInput")
    wtv_d = nc.dram_tensor("wtv", [C, F], F32R, kind="ExternalInput")
    bq_d = nc.dram_tensor("bq", [F], F32, kind="ExternalInput")
    bk_d = nc.dram_tensor("bk", [1, F], BF16, kind="ExternalInput")
    bv_d = nc.dram_tensor("bv", [1, F], BF16, kind="ExternalInput")
    mq_d = nc.dram_tensor("mq", [L], F32, kind="ExternalInput")
    mkv_d = nc.dram_tensor("mkv", [L], F32, kind="ExternalInput")
    out_d = nc.dram_tensor("out", [L, F], F32, kind="ExternalOutput")

    with tile.TileContext(nc) as tc:
        with (
            tc.tile_pool(name="singles", bufs=1) as singles,
            tc.tile_pool(name="xtp", bufs=3) as xtp,
            tc.tile_pool(name="tmp", bufs=3) as tmp,
            tc.tile_pool(name="kvout", bufs=3) as kvout,
        ):
            # constants / weights
            wtq_sb = singles.tile([128, KC, F], F32R)
            nc.sync.dma_start(wtq_sb[:], wtq_d.ap().rearrange("(o p) f -> p o f", p=128))
            wtk_sb = singles.tile([128, KC, F], F32R)
            nc.sync.dma_start(wtk_sb[:], wtk_d.ap().rearrange("(o p) f -> p o f", p=128))
            wtv_sb = singles.tile([128, KC, F], F32R)
            nc.sync.dma_start(wtv_sb[:], wtv_d.ap().rearrange("(o p) f -> p o f", p=128))
            bq_pp = singles.tile([128, FT], F32)
            nc.sync.dma_start(bq_pp[:], bq_d.ap().rearrange("(f p) -> p f", p=128))
            bk_row = singles.tile([1, F], BF16)
            nc.sync.dma_start(bk_row[:], bk_d.ap())
            bv_row = singles.tile([1, F], BF16)
            nc.sync.dma_start(bv_row[:], bv_d.ap())
            ones_row = singles.tile([1, 128], BF16)
            nc.vector.memset(ones_row[:], 1.0)
            mq_sb = singles.tile([128, NST], F32)
            nc.sync.dma_start(mq_sb[:], mq_d.ap().rearrange("(t p) -> p t", p=128))
            mkv_sb = singles.tile([128, NST], F32)
            nc.sync.dma_start(mkv_sb[:], mkv_d.ap().rearrange("(t p) -> p t", p=128))
            # derived mask tiles
            mm2L = singles.tile([128, NST], F32)   # m_kv^2 / L
            nc.vector.scalar_tensor_tensor(mm2L[:], mkv_sb[:], 1.0 / L, mkv_sb[:],
                                           ALU.mult, ALU.mult)
            mqL = singles.tile([128, NST], F32)    # m_q * L
            nc.vector.tensor_scalar(mqL[:], mq_sb[:], float(L), None, ALU.mult)
            mkv8 = singles.tile([128, NST, HPC], F32)  # mask col replicated per head
            for h in range(HPC):
                nc.vector.tensor_copy(out=mkv8[:, :, h:h + 1], in_=mkv_sb[:, :, None])

            QT = singles.tile([128, FT, L], F32)   # resident Q^T (feat-major)

            xtq_ap = xtq_d.ap().rearrange("(o p) l -> p o l", p=128)
            xtk_ap = xtk_d.ap().rearrange("(o p) l -> p o l", p=128)
            xtv_ap = xtv_d.ap().rearrange("(o p) l -> p o l", p=128)

            with tc.tile_pool(name="psA", bufs=2, space="PSUM") as psA:
                kvA = psA.tile([64, FT, 65], F32, name="kvA", bufs=1)
                kvB = psA.tile([64, FT, 65], F32, name="kvB", bufs=1)
                for b in range(NBLK):
                    xtq_t = xtp.tile([128, KC, 512], F32R, tag="xt")
                    nc.sync.dma_start(xtq_t[:], xtq_ap[:, :, ds(b * 512, 512)])
                    xtk_t = xtp.tile([128, KC, 512], F32R, tag="xt")
                    nc.sync.dma_start(xtk_t[:], xtk_ap[:, :, ds(b * 512, 512)])
                    xtv_t = xtp.tile([128, KC, 512], F32R, tag="xt")
                    nc.sync.dma_start(xtv_t[:], xtv_ap[:, :, ds(b * 512, 512)])

                    # ---- Q^T tiles: (128 feat, 512 tok), weights stationary
                    for f in range(FT):
                        psq = psA.tile([128, 512], F32, tag="psq")
                        for k in range(KC):
                            nc.tensor.matmul(psq[:], wtq_sb[:, k, ts(f, 128)],
                                             xtq_t[:, k, :],
                                             start=(k == 0), stop=(k == KC - 1))
                        E = tmp.tile([128, 512], F32, tag="E")
                        nc.scalar.activation(E[:], psq[:], ACTF.Exp,
                                             bias=bq_pp[:, f:f + 1])
                        R = tmp.tile([128, 512], F32, tag="R")
                        nc.scalar.activation(R[:], psq[:], ACTF.Relu,
                                             bias=bq_pp[:, f:f + 1])
                        nc.vector.scalar_tensor_tensor(
                            QT[:, f, ds(b * 512, 512)], E[:], 1.0, R[:],
                            ALU.min, ALU.add)

                    # ---- K, V, kv per 128-token subtile
                    for s in range(4):
                        st = b * 4 + s
                        psk = psA.tile([128, 512], F32, tag="psk")
                        for k in range(KC):
                            nc.tensor.matmul(psk[:], xtk_t[:, k, ts(s, 128)],
                                             wtk_sb[:, k, :],
                                             start=(k == 0), stop=False)
                        nc.tensor.matmul(psk[:], ones_row[:], bk_row[:],
                                         start=False, stop=True)
                        E2 = tmp.tile([128, 512], F32, tag="E")
                        nc.scalar.activation(E2[:], psk[:], ACTF.Exp)
                        R2 = tmp.tile([128, 512], F32, tag="R")
                        nc.scalar.activation(R2[:], psk[:], ACTF.Relu)
                        Ksb = tmp.tile([128, 512], F32, tag="Ksb", bufs=2)
                        nc.vector.scalar_tensor_tensor(Ksb[:], E2[:], 1.0, R2[:],
                                                       ALU.min, ALU.add)

                        psv = psA.tile([128, 512], F32, tag="psv")
                        for k in range(KC):
                            nc.tensor.matmul(psv[:], xtv_t[:, k, ts(s, 128)],
                                             wtv_sb[:, k, :],
                                             start=(k == 0), stop=False)
                        nc.tensor.matmul(psv[:], ones_row[:], bv_row[:],
                                         start=False, stop=True)
                        V1 = tmp.tile([128, HPC, 65], F32, tag="V1", bufs=2)
                        nc.vector.tensor_scalar(
                            V1[:, :, 0:64],
                            psv.rearrange("p (h d) -> p h d", d=64),
                            mm2L[:, st:st + 1], None, ALU.mult)
                        nc.vector.tensor_copy(out=V1[:, :, 64:65],
                                              in_=mkv8[:, st, :, None])

                        for h in range(HPC):
                            kvp = kvA if h % 2 == 0 else kvB
                            nc.tensor.matmul(
                                kvp[:, h // 2, :],
                                Ksb[:, ts(h, 64)],
                                V1[:, h, :],
                                start=(st == 0 and h < 2),
                                stop=(st == NST - 1 and h // 2 == FT - 1),
                                skip_group_check=True)

                # block-diagonal kv for the output einsum:
                # rows 0:64 = even head of pair (cols 0:65), rows 64:128 = odd
                kv_sb = singles.tile([128, FT, 130], F32)
                nc.vector.memset(kv_sb[:], 0.0)
                nc.vector.tensor_copy(out=kv_sb[0:64, :, 0:65], in_=kvA[:])
                nc.vector.tensor_copy(out=kv_sb[64:128, :, 65:130], in_=kvB[:])

            with tc.tile_pool(name="psB", bufs=2, space="PSUM") as psB:
                for st in range(NST):
                    poA = psB.tile([128, 2, 130], F32, tag="poA")
                    poB = psB.tile([128, 2, 130], F32, tag="poB")
                    for f in range(FT):
                        po = poA if f < 2 else poB
                        nc.tensor.matmul(po[:, f % 2, :],
                                         QT[:, f, ts(st, 128)],
                                         kv_sb[:, f, :],
                                         start=True, stop=True,
                                         skip_group_check=True)
                    qs8 = kvout.tile([128, HPC], F32, tag="qs8")
                    nc.vector.tensor_copy(
                        out=qs8[:, 0:4].rearrange("p (f c) -> p f c", c=2),
                        in_=poA[:, :, 64:130:65])
                    nc.vector.tensor_copy(
                        out=qs8[:, 4:8].rearrange("p (f c) -> p f c", c=2),
                        in_=poB[:, :, 64:130:65])
                    den = kvout.tile([128, HPC], F32, tag="den")
                    nc.vector.tensor_scalar(den[:], qs8[:], mq_sb[:, st:st + 1],
                                            EPS, ALU.mult, ALU.add)
                    rcp = kvout.tile([128, HPC], F32, tag="rcp")
                    nc.vector.reciprocal(rcp[:], den[:])
                    s8 = kvout.tile([128, HPC], F32, tag="s8")
                    nc.vector.tensor_scalar(s8[:], rcp[:], mqL[:, st:st + 1],
                                            None, ALU.mult)
                    outsb = kvout.tile([128, HPC, 64], F32, tag="outsb")
                    nc.vector.tensor_tensor(
                        out=outsb[:, 0:4:2, :], in0=poA[:, :, 0:64],
                        in1=s8[:, 0:4:2].to_broadcast((128, 2, 64)), op=ALU.mult)
                    nc.vector.tensor_tensor(
                        out=outsb[:, 1:4:2, :], in0=poA[:, :, 65:129],
                        in1=s8[:, 1:4:2].to_broadcast((128, 2, 64)), op=ALU.mult)
                    nc.vector.tensor_tensor(
                        out=outsb[:, 4:8:2, :], in0=poB[:, :, 0:64],
                        in1=s8[:, 4:8:2].to_broadcast((128, 2, 64)), op=ALU.mult)
                    nc.vector.tensor_tensor(
                        out=outsb[:, 5:8:2, :], in0=poB[:, :, 65:129],
                        in1=s8[:, 5:8:2].to_broadcast((128, 2, 64)), op=ALU.mult)
                    nc.sync.dma_start(out_d.ap()[ds(st * 128, 128), :],
                                      outsb.rearrange("p h d -> p (h d)"))

    nc.compile()
    return nc


def _get_program():
    global _PROG
    if _PROG is None:
        _PROG = _build_program()
    return _PROG


def kernel(query, key, value, mask_q, mask_kv, Wq, bq, Wk, bk, Wv, bv):
    query = np.asarray(query, dtype=np.float32)
    key = np.asarray(key, dtype=np.float32)
    value = np.asarray(value, dtype=np.float32)
    mask_q = np.asarray(mask_q, dtype=np.float32)
    mask_kv = np.asarray(mask_kv, dtype=np.float32)
    Wq = np.asarray(Wq, dtype=np.float32)
    bq = np.asarray(bq, dtype=np.float32)
    Wk = np.asarray(Wk, dtype=np.float32)
    bk = np.asarray(bk, dtype=np.float32)
    Wv = np.asarray(Wv, dtype=np.float32)
    bv = np.asarray(bv, dtype=np.float32)

    nc = _get_program()

    xt = {}
    for b in range(B):
        xt[b] = (np.ascontiguousarray(query[b].T),
                 np.ascontiguousarray(key[b].T),
                 np.ascontiguousarray(value[b].T))
    wslices = {}
    for hh in range(2):
        sl = slice(hh * F, (hh + 1) * F)
        wslices[hh] = (
            np.ascontiguousarray(Wq[sl, :].T),
            np.ascontiguousarray(Wk[sl, :].T),
            np.ascontiguousarray(Wv[sl, :].T),
            bq[sl].copy(),
            bk[sl].astype(ml_dtypes.bfloat16).reshape(1, F),
            bv[sl].astype(ml_dtypes.bfloat16).reshape(1, F),
        )

    in_maps = []
    for core in range(NCORES):
        b, hh = core // 2, core % 2
        xtq, xtk, xtv = xt[b]
        wtq, wtk, wtv, bq_h, bk_h, bv_h = wslices[hh]
        in_maps.append({
            "xtq": xtq, "xtk": xtk, "xtv": xtv,
            "wtq": wtq, "wtk": wtk, "wtv": wtv,
            "bq": bq_h, "bk": bk_h, "bv": bv_h,
            "mq": mask_q[b], "mkv": mask_kv[b],
        })

    trace = os.environ.get("TRN_KERNEL_TRACE", "0") == "1"
    trace_cores = list(range(NCORES)) if trace else None
    res = run_bass_kernel_spmd(nc, in_maps, list(range(NCORES)),
                               trace=trace, trace_cores=trace_cores)
    if trace:
        kernel.last_exec_time_ns = res.exec_time_ns
        kernel.last_scope_times = res.per_core_scope_times

    out = np.empty((B, L, H, D), dtype=np.float32)
    for core in range(NCORES):
        b, hh = core // 2, core % 2
        out[b, :, hh * HPC:(hh + 1) * HPC, :] = \
            res.results[core]["out"].reshape(L, HPC, D)
    return out


# revision 5
# speedup vs baseline: 1.0938x; 1.0938x over previous
"""Trainium2 Bass kernel for nn_MultiHeadLinearAttention.

Sharding: batch (4-way) x head-half (2-way) = 8 cores, no cross-core
reductions (the kv / summed_k contractions are over L, which stays local).

Per core (b = core//2, hh = core%2, 8 heads, F=512 local feature cols):
  phase A (per 512-token block):
    Q^T = fm(Wq_h @ X_q^T + bq)          (feat-major, f32r matmuls, N=512)
    K   = fm(X_k Wk_h^T + bk)            (token-major; bias via rank-1 matmul)
    V1  = [(X_v Wv_h^T + bv) * m_kv^2/L | m_kv]
    kv[h] += K_h^T @ V1_h                (fp32, accumulated in PSUM all L)
  phase B (per 128-token tile): out = (Q^T)^T @ blockdiag(kv) scaled by
    m_q*L / (eps + m_q * (q . summed_k)), summed_k taken from the mask
    column of kv.
fm(y) = elu(y)+1 = min(exp(y),1) + relu(y).
"""

import os
import sys
import types

for _p in ("/opt/trn_rl_repo",):
    if _p not in sys.path:
        sys.path.insert(0, _p)
os.environ.setdefault("MYCRO_LOCAL_CACHE", "1")

import numpy as np
import ml_dtypes

# Optional NTFF profiling support (used by test harness via TRN_KERNEL_TRACE=1).
if "antenv.axon_hooks" not in sys.modules:
    _hook_mod = types.ModuleType("antenv.axon_hooks")
    _hook_state = {"h": None}
    _hook_mod.set_axon_ntff_profile_hook = lambda h: _hook_state.__setitem__("h", h)
    _hook_mod.get_axon_ntff_profile_hook = lambda: _hook_state["h"]
    sys.modules["antenv.axon_hooks"] = _hook_mod
    try:
        from trn_agent_boot.trn_boot import _ntff_profile_via_ctypes
        _hook_state["h"] = _ntff_profile_via_ctypes("/opt/axon/libaxon_pjrt.so")
    except Exception:
        pass

import concourse.bacc as bacc
import concourse.tile as tile
import concourse.mybir as mybir
import concourse.bass_utils as bass_utils
from concourse.bass import ts, ds
from concourse.bass_utils import run_bass_kernel_spmd

bass_utils.upload_artifacts = lambda tmpdir: tmpdir  # no artifact bucket here

F32 = mybir.dt.float32
F32R = mybir.dt.float32r
BF16 = mybir.dt.bfloat16
ACTF = mybir.ActivationFunctionType
ALU = mybir.AluOpType

B, L, C, H, D = 4, 4096, 1024, 16, 64
EPS = 1e-6
NCORES = 8
HPC = H // NCORES * 4          # 8 heads per core
F = HPC * D                    # 512 local feature cols
KC = C // 128                  # 8 contraction chunks
NBLK = L // 512                # 8 token blocks (phase A)
NST = L // 128                 # 32 token subtiles
FT = F // 128                  # 4 feature tiles (head pairs)

_PROG = None


def _build_program():
    nc = bacc.Bacc("TRN2", target_bir_lowering=False, debug=False,
                   num_devices=NCORES)

    xtq_d = nc.dram_tensor("xtq", [C, L], F32R, kind="ExternalInput")
    xtk_d = nc.dram_tensor("xtk", [C, L], F32R, kind="ExternalInput")
    xtv_d = nc.dram_tensor("xtv", [C, L], F32R, kind="ExternalInput")
    wtq_d = nc.dram_tensor("wtq", [C, F], F32R, kind="ExternalInput")
    wtk_d = nc.dram_tensor("wtk", [C, F], F32R, kind="ExternalInput")
    wtv_d = nc.dram_tensor("wtv", [C, F], F32R, kind="ExternalInput")
    bq_d = nc.dram_tensor("bq", [F], F32, kind="ExternalInput")
    bk_d = nc.dram_tensor("bk", [1, F], BF16, kind="ExternalInput")
    bv_d = nc.dram_tensor("bv", [1, F], BF16, kind="ExternalInput")
    mq_d = nc.dram_tensor("mq", [L], F32, kind="ExternalInput")
    mkv_d = nc.dram_tensor("mkv", [L], F32, kind="ExternalInput")
    out_d = nc.dram_tensor("out", [L, F], F32, kind="ExternalOutput")

    with tile.TileContext(nc) as tc:
        with (
            tc.tile_pool(name="singles", bufs=1) as singles,
            tc.tile_pool(name="xtp", bufs=3) as xtp,
            tc.tile_pool(name="tmp", bufs=3) as tmp,
            tc.tile_pool(name="kvout", bufs=3) as kvout,
        ):
            # constants / weights
            wtq_sb = singles.tile([128, KC, F], F32R)
            nc.sync.dma_start(wtq_sb[:], wtq_d.ap().rearrange("(o p) f -> p o f", p=128))
            wtk_sb = singles.tile([128, KC, F], F32R)
            nc.sync.dma_start(wtk_sb[:], wtk_d.ap().rearrange("(o p) f -> p o f", p=128))
            wtv_sb = singles.tile([128, KC, F], F32R)
            nc.sync.dma_start(wtv_sb[:], wtv_d.ap().rearrange("(o p) f -> p o f", p=128))
            bq_pp = singles.tile([128, FT], F32)
            nc.sync.dma_start(bq_pp[:], bq_d.ap().rearrange("(f p) -> p f", p=128))
            bk_row = singles.tile([1, F], BF16)
            nc.sync.dma_start(bk_row[:], bk_d.ap())
            bv_row = singles.tile([1, F], BF16)
            nc.sync.dma_start(bv_row[:], bv_d.ap())
            ones_row = singles.tile([1, 128], BF16)
            nc.vector.memset(ones_row[:], 1.0)
            mq_sb = singles.tile([128, NST], F32)
            nc.sync.dma_start(mq_sb[:], mq_d.ap().rearrange("(t p) -> p t", p=128))
            mkv_sb = singles.tile([128, NST], F32)
            nc.sync.dma_start(mkv_sb[:], mkv_d.ap().rearrange("(t p) -> p t", p=128))
            # derived mask tiles
            mm2L = singles.tile([128, NST], F32)   # m_kv^2 / L
            nc.vector.scalar_tensor_tensor(mm2L[:], mkv_sb[:], 1.0 / L, mkv_sb[:],
                                           ALU.mult, ALU.mult)
            mqL = singles.tile([128, NST], F32)    # m_q * L
            nc.vector.tensor_scalar(mqL[:], mq_sb[:], float(L), None, ALU.mult)
            mkv8 = singles.tile([128, NST, HPC], F32)  # mask col replicated per head
            for h in range(HPC):
                nc.vector.tensor_copy(out=mkv8[:, :, h:h + 1], in_=mkv_sb[:, :, None])

            QT = singles.tile([128, FT, L], BF16)  # resident Q^T (feat-major)

            xtq_ap = xtq_d.ap().rearrange("(o p) l -> p o l", p=128)
            xtk_ap = xtk_d.ap().rearrange("(o p) l -> p o l", p=128)
            xtv_ap = xtv_d.ap().rearrange("(o p) l -> p o l", p=128)

            with tc.tile_pool(name="psA", bufs=2, space="PSUM") as psA:
                kvA = psA.tile([64, FT, 65], F32, name="kvA", bufs=1)
                kvB = psA.tile([64, FT, 65], F32, name="kvB", bufs=1)
                for b in range(NBLK):
                    with nc.named_scope(f"blk{b}"):
                        xtq_t = xtp.tile([128, KC, 512], F32R, tag="xt")
                        nc.sync.dma_start(xtq_t[:], xtq_ap[:, :, ds(b * 512, 512)])
                        xtk_t = xtp.tile([128, KC, 512], F32R, tag="xt")
                        nc.sync.dma_start(xtk_t[:], xtk_ap[:, :, ds(b * 512, 512)])
                        xtv_t = xtp.tile([128, KC, 512], F32R, tag="xt")
                        nc.sync.dma_start(xtv_t[:], xtv_ap[:, :, ds(b * 512, 512)])

                        # ---- K projections + feature map, 4 subtiles
                        Ksbs, V1s = [], []
                        for s in range(4):
                            psk = psA.tile([128, 512], F32, tag="psk")
                            for k in range(KC):
                                nc.tensor.matmul(psk[:], xtk_t[:, k, ts(s, 128)],
                                                 wtk_sb[:, k, :],
                                                 start=(k == 0), stop=False)
                            nc.tensor.matmul(psk[:], ones_row[:], bk_row[:],
                                             start=False, stop=True)
                            E2 = tmp.tile([128, 512], F32, tag="E")
                            nc.scalar.activation(E2[:], psk[:], ACTF.Exp)
                            R2 = tmp.tile([128, 512], F32, tag="R")
                            nc.scalar.activation(R2[:], psk[:], ACTF.Relu)
                            Ksb = tmp.tile([128, 512], BF16, tag="Ksb", bufs=5)
                            nc.vector.scalar_tensor_tensor(Ksb[:], E2[:], 1.0,
                                                           R2[:], ALU.min, ALU.add)
                            Ksbs.append(Ksb)

                        # ---- V projections + mask/scale, 4 subtiles
                        for s in range(4):
                            st = b * 4 + s
                            psv = psA.tile([128, 512], F32, tag="psv")
                            for k in range(KC):
                                nc.tensor.matmul(psv[:], xtv_t[:, k, ts(s, 128)],
                                                 wtv_sb[:, k, :],
                                                 start=(k == 0), stop=False)
                            nc.tensor.matmul(psv[:], ones_row[:], bv_row[:],
                                             start=False, stop=True)
                            V1 = tmp.tile([128, HPC, 65], BF16, tag="V1", bufs=5)
                            nc.vector.tensor_scalar(
                                V1[:, :, 0:64],
                                psv.rearrange("p (h d) -> p h d", d=64),
                                mm2L[:, st:st + 1], None, ALU.mult)
                            nc.vector.tensor_copy(out=V1[:, :, 64:65],
                                                  in_=mkv8[:, st, :, None])
                            V1s.append(V1)

                        # ---- Q^T tiles: (128 feat, 512 tok), weights stationary
                        for f in range(FT):
                            psq = psA.tile([128, 512], F32, tag="psq")
                            for k in range(KC):
                                nc.tensor.matmul(psq[:], wtq_sb[:, k, ts(f, 128)],
                                                 xtq_t[:, k, :],
                                                 start=(k == 0), stop=(k == KC - 1))
                            E = tmp.tile([128, 512], F32, tag="E")
                            nc.scalar.activation(E[:], psq[:], ACTF.Exp,
                                                 bias=bq_pp[:, f:f + 1])
                            R = tmp.tile([128, 512], F32, tag="R")
                            nc.scalar.activation(R[:], psq[:], ACTF.Relu,
                                                 bias=bq_pp[:, f:f + 1])
                            nc.vector.scalar_tensor_tensor(
                                QT[:, f, ds(b * 512, 512)], E[:], 1.0, R[:],
                                ALU.min, ALU.add)

                        # ---- kv accumulation, 4 subtiles x 8 heads
                        for s in range(4):
                            st = b * 4 + s
                            for h in range(HPC):
                                kvp = kvA if h % 2 == 0 else kvB
                                nc.tensor.matmul(
                                    kvp[:, h // 2, :],
                                    Ksbs[s][:, ts(h, 64)],
                                    V1s[s][:, h, :],
                                    start=(st == 0 and h < 2),
                                    stop=(st == NST - 1 and h // 2 == FT - 1),
                                    skip_group_check=True)

                # block-diagonal kv for the output einsum:
                # rows 0:64 = even head of pair (cols 0:65), rows 64:128 = odd
                kv_sb = singles.tile([128, FT, 130], BF16)
                nc.vector.memset(kv_sb[:], 0.0)
                nc.vector.tensor_copy(out=kv_sb[0:64, :, 0:65], in_=kvA[:])
                nc.vector.tensor_copy(out=kv_sb[64:128, :, 65:130], in_=kvB[:])

            with tc.tile_pool(name="psB", bufs=2, space="PSUM") as psB, \
                 nc.named_scope("phaseB"):
                for st in range(NST):
                    poA = psB.tile([128, 2, 130], F32, tag="poA")
                    poB = psB.tile([128, 2, 130], F32, tag="poB")
                    for f in range(FT):
                        po = poA if f < 2 else poB
                        nc.tensor.matmul(po[:, f % 2, :],
                                         QT[:, f, ts(st, 128)],
                                         kv_sb[:, f, :],
                                         start=True, stop=True,
                                         skip_group_check=True)
                    qs8 = kvout.tile([128, HPC], F32, tag="qs8")
                    nc.vector.tensor_copy(
                        out=qs8[:, 0:4].rearrange("p (f c) -> p f c", c=2),
                        in_=poA[:, :, 64:130:65])
                    nc.vector.tensor_copy(
                        out=qs8[:, 4:8].rearrange("p (f c) -> p f c", c=2),
                        in_=poB[:, :, 64:130:65])
                    den = kvout.tile([128, HPC], F32, tag="den")
                    nc.vector.tensor_scalar(den[:], qs8[:], mq_sb[:, st:st + 1],
                                            EPS, ALU.mult, ALU.add)
                    rcp = kvout.tile([128, HPC], F32, tag="rcp")
                    nc.vector.reciprocal(rcp[:], den[:])
                    s8 = kvout.tile([128, HPC], F32, tag="s8")
                    nc.vector.tensor_scalar(s8[:], rcp[:], mqL[:, st:st + 1],
                                            None, ALU.mult)
                    outsb = kvout.tile([128, HPC, 64], F32, tag="outsb")
                    # heads 0-3 scaled on ScalarE (idle in phase B), 4-7 on DVE
                    for h in range(4):
                        f, par = h // 2, h % 2
                        cols = slice(0, 64) if par == 0 else slice(65, 129)
                        nc.scalar.activation(outsb[:, h, :], poA[:, f, cols],
                                             ACTF.Identity, bias=0.0,
                                             scale=s8[:, h:h + 1])
                    nc.vector.tensor_tensor(
                        out=outsb[:, 4:8:2, :], in0=poB[:, :, 0:64],
                        in1=s8[:, 4:8:2].to_broadcast((128, 2, 64)), op=ALU.mult)
                    nc.vector.tensor_tensor(
                        out=outsb[:, 5:8:2, :], in0=poB[:, :, 65:129],
                        in1=s8[:, 5:8:2].to_broadcast((128, 2, 64)), op=ALU.mult)
                    nc.sync.dma_start(out_d.ap()[ds(st * 128, 128), :],
                                      outsb.rearrange("p h d -> p (h d)"))

    nc.compile()
    return nc


def _get_program():
    global _PROG
    if _PROG is None:
        _PROG = _build_program()
    return _PROG


def kernel(query, key, value, mask_q, mask_kv, Wq, bq, Wk, bk, Wv, bv):
    query = np.asarray(query, dtype=np.float32)
    key = np.asarray(key, dtype=np.float32)
    value = np.asarray(value, dtype=np.float32)
    mask_q = np.asarray(mask_q, dtype=np.float32)
    mask_kv = np.asarray(mask_kv, dtype=np.float32)
    Wq = np.asarray(Wq, dtype=np.float32)
    bq = np.asarray(bq, dtype=np.float32)
    Wk = np.asarray(Wk, dtype=np.float32)
    bk = np.asarray(bk, dtype=np.float32)
    Wv = np.asarray(Wv, dtype=np.float32)
    bv = np.asarray(bv, dtype=np.float32)

    nc = _get_program()

    xt = {}
    for b in range(B):
        xt[b] = (np.ascontiguousarray(query[b].T),
                 np.ascontiguousarray(key[b].T),
                 np.ascontiguousarray(value[b].T))
    wslices = {}
    for hh in range(2):
        sl = slice(hh * F, (hh + 1) * F)
        wslices[hh] = (
            np.ascontiguousarray(Wq[sl, :].T),
            np.ascontiguousarray(Wk[sl, :].T),
            np.ascontiguousarray(Wv[sl, :].T),
            bq[sl].copy(),
            bk[sl].astype(ml_dtypes.bfloat16).reshape(1, F),
            bv[sl].astype(ml_dtypes.bfloat16).reshape(1, F),
        )

    in_maps = []
    for core in range(NCORES):
        b, hh = core // 2, core % 2
        xtq, xtk, xtv = xt[b]
        wtq, wtk, wtv, bq_h, bk_h, bv_h = wslices[hh]
        in_maps.append({
            "xtq": xtq, "xtk": xtk, "xtv": xtv,
            "wtq": wtq, "wtk": wtk, "wtv": wtv,
            "bq": bq_h, "bk": bk_h, "bv": bv_h,
            "mq": mask_q[b], "mkv": mask_kv[b],
        })

    trace = os.environ.get("TRN_KERNEL_TRACE", "0") == "1"
    trace_cores = list(range(NCORES)) if trace else None
    res = run_bass_kernel_spmd(nc, in_maps, list(range(NCORES)),
                               trace=trace, trace_cores=trace_cores)
    if trace:
        kernel.last_exec_time_ns = res.exec_time_ns
        kernel.last_scope_times = res.per_core_scope_times

    out = np.empty((B, L, H, D), dtype=np.float32)
    for core in range(NCORES):
        b, hh = core // 2, core % 2
        out[b, :, hh * HPC:(hh + 1) * HPC, :] = \
            res.results[core]["out"].reshape(L, HPC, D)
    return out


# revision 10
# speedup vs baseline: 1.1612x; 1.0616x over previous
"""Trainium2 Bass kernel for nn_MultiHeadLinearAttention.

Sharding: batch (4-way) x head-half (2-way) = 8 cores, no cross-core
reductions (the kv / summed_k contractions are over L, which stays local).

Per core (b = core//2, hh = core%2, 8 heads, F=512 local feature cols):
  phase A (per 512-token block):
    Q^T = fm(Wq_h @ X_q^T + bq)          (feat-major, f32r matmuls, N=512)
    K   = fm(X_k Wk_h^T + bk)            (token-major; bias via rank-1 matmul)
    V1  = [(X_v Wv_h^T + bv) * m_kv^2/L | m_kv]
    kv[h] += K_h^T @ V1_h                (fp32, accumulated in PSUM all L)
  phase B (per 128-token tile): out = (Q^T)^T @ blockdiag(kv) scaled by
    m_q*L / (eps + m_q * (q . summed_k)), summed_k taken from the mask
    column of kv.
fm(y) = elu(y)+1 = min(exp(y),1) + relu(y).
"""

import os
import sys
import types

for _p in ("/opt/trn_rl_repo",):
    if _p not in sys.path:
        sys.path.insert(0, _p)
os.environ.setdefault("MYCRO_LOCAL_CACHE", "1")

import numpy as np
import ml_dtypes

# Optional NTFF profiling support (used by test harness via TRN_KERNEL_TRACE=1).
if "antenv.axon_hooks" not in sys.modules:
    _hook_mod = types.ModuleType("antenv.axon_hooks")
    _hook_state = {"h": None}
    _hook_mod.set_axon_ntff_profile_hook = lambda h: _hook_state.__setitem__("h", h)
    _hook_mod.get_axon_ntff_profile_hook = lambda: _hook_state["h"]
    sys.modules["antenv.axon_hooks"] = _hook_mod
    try:
        from trn_agent_boot.trn_boot import _ntff_profile_via_ctypes
        _hook_state["h"] = _ntff_profile_via_ctypes("/opt/axon/libaxon_pjrt.so")
    except Exception:
        pass

import concourse.bacc as bacc
import concourse.tile as tile
import concourse.mybir as mybir
import concourse.bass_utils as bass_utils
from concourse.bass import ts, ds
from concourse.bass_utils import run_bass_kernel_spmd

bass_utils.upload_artifacts = lambda tmpdir: tmpdir  # no artifact bucket here

F32 = mybir.dt.float32
F32R = mybir.dt.float32r
BF16 = mybir.dt.bfloat16
ACTF = mybir.ActivationFunctionType
ALU = mybir.AluOpType

B, L, C, H, D = 4, 4096, 1024, 16, 64
EPS = 1e-6
NCORES = 8
HPC = H // NCORES * 4          # 8 heads per core
F = HPC * D                    # 512 local feature cols
KC = C // 128                  # 8 contraction chunks
NBLK = L // 512                # 8 token blocks (phase A)
NST = L // 128                 # 32 token subtiles
FT = F // 128                  # 4 feature tiles (head pairs)

_PROG = None


def _build_program():
    nc = bacc.Bacc("TRN2", target_bir_lowering=False, debug=False,
                   num_devices=NCORES)

    xtq_d = nc.dram_tensor("xtq", [C, L], F32R, kind="ExternalInput")
    xtk_d = nc.dram_tensor("xtk", [C, L], F32R, kind="ExternalInput")
    xtv_d = nc.dram_tensor("xtv", [C, L], F32R, kind="ExternalInput")
    wtq_d = nc.dram_tensor("wtq", [C, F], F32R, kind="ExternalInput")
    wtk_d = nc.dram_tensor("wtk", [C, F], F32R, kind="ExternalInput")
    wtv_d = nc.dram_tensor("wtv", [C, F], F32R, kind="ExternalInput")
    bq_d = nc.dram_tensor("bq", [F], F32, kind="ExternalInput")
    bk_d = nc.dram_tensor("bk", [1, F], BF16, kind="ExternalInput")
    bv_d = nc.dram_tensor("bv", [1, F], BF16, kind="ExternalInput")
    mq_d = nc.dram_tensor("mq", [L], F32, kind="ExternalInput")
    mkv_d = nc.dram_tensor("mkv", [L], F32, kind="ExternalInput")
    out_d = nc.dram_tensor("out", [L, F], F32, kind="ExternalOutput")

    with tile.TileContext(nc) as tc:
        with (
            tc.tile_pool(name="singles", bufs=1) as singles,
            tc.tile_pool(name="xtp", bufs=4) as xtp,
            tc.tile_pool(name="tmp", bufs=3) as tmp,
            tc.tile_pool(name="kvout", bufs=3) as kvout,
        ):
            # constants / weights.  DMA order matters for the pipeline head:
            # the first matmuls need (wtk, xtk block 0) — load those first.
            xtq_ap = xtq_d.ap().rearrange("(o p) l -> p o l", p=128)
            xtk_ap = xtk_d.ap().rearrange("(o p) l -> p o l", p=128)
            xtv_ap = xtv_d.ap().rearrange("(o p) l -> p o l", p=128)

            bk_row = singles.tile([1, F], BF16)
            nc.sync.dma_start(bk_row[:], bk_d.ap())
            wtk_sb = singles.tile([128, KC, F], F32R)
            nc.sync.dma_start(wtk_sb[:], wtk_d.ap().rearrange("(o p) f -> p o f", p=128))
            xtk_t0 = xtp.tile([128, KC, 512], F32R, tag="xt")
            nc.sync.dma_start(xtk_t0[:], xtk_ap[:, :, ds(0, 512)])
            bv_row = singles.tile([1, F], BF16)
            nc.sync.dma_start(bv_row[:], bv_d.ap())
            wtv_sb = singles.tile([128, KC, F], F32R)
            nc.sync.dma_start(wtv_sb[:], wtv_d.ap().rearrange("(o p) f -> p o f", p=128))
            xtv_t0 = xtp.tile([128, KC, 512], F32R, tag="xt")
            nc.sync.dma_start(xtv_t0[:], xtv_ap[:, :, ds(0, 512)])
            bq_pp = singles.tile([128, FT], F32)
            nc.sync.dma_start(bq_pp[:], bq_d.ap().rearrange("(f p) -> p f", p=128))
            wtq_sb = singles.tile([128, KC, F], F32R)
            nc.sync.dma_start(wtq_sb[:], wtq_d.ap().rearrange("(o p) f -> p o f", p=128))
            xtq_t0 = xtp.tile([128, KC, 512], F32R, tag="xt")
            nc.sync.dma_start(xtq_t0[:], xtq_ap[:, :, ds(0, 512)])
            ones_row = singles.tile([1, 128], BF16)
            nc.vector.memset(ones_row[:], 1.0)
            mq_sb = singles.tile([128, NST], F32)
            nc.sync.dma_start(mq_sb[:], mq_d.ap().rearrange("(t p) -> p t", p=128))
            mkv_sb = singles.tile([128, NST], F32)
            nc.sync.dma_start(mkv_sb[:], mkv_d.ap().rearrange("(t p) -> p t", p=128))
            # derived mask tiles
            mm2L = singles.tile([128, NST], F32)   # m_kv^2 / L
            nc.vector.scalar_tensor_tensor(mm2L[:], mkv_sb[:], 1.0 / L, mkv_sb[:],
                                           ALU.mult, ALU.mult)
            mqL = singles.tile([128, NST], F32)    # m_q * L
            nc.vector.tensor_scalar(mqL[:], mq_sb[:], float(L), None, ALU.mult)
            mkv8 = singles.tile([128, NST, HPC], F32)  # mask col replicated per head
            for h in range(HPC):
                nc.vector.tensor_copy(out=mkv8[:, :, h:h + 1], in_=mkv_sb[:, :, None])

            QT = singles.tile([128, FT, L], BF16)  # resident Q^T (feat-major)

            with tc.tile_pool(name="psA", bufs=2, space="PSUM") as psA:
                kvA = psA.tile([64, FT, 65], F32, name="kvA", bufs=1)
                kvB = psA.tile([64, FT, 65], F32, name="kvB", bufs=1)
                for b in range(NBLK):
                    with nc.named_scope(f"blk{b}"):
                        if b == 0:
                            xtk_t, xtv_t, xtq_t = xtk_t0, xtv_t0, xtq_t0
                        else:
                            xtk_t = xtp.tile([128, KC, 512], F32R, tag="xt")
                            nc.sync.dma_start(xtk_t[:], xtk_ap[:, :, ds(b * 512, 512)])
                            xtv_t = xtp.tile([128, KC, 512], F32R, tag="xt")
                            nc.sync.dma_start(xtv_t[:], xtv_ap[:, :, ds(b * 512, 512)])
                            xtq_t = xtp.tile([128, KC, 512], F32R, tag="xt")
                            nc.sync.dma_start(xtq_t[:], xtq_ap[:, :, ds(b * 512, 512)])

                        # ---- K projections + feature map, 4 subtiles
                        Ksbs, V1s = [], []
                        for s in range(4):
                            psk = psA.tile([128, 512], F32, tag="psk")
                            for k in range(KC):
                                nc.tensor.matmul(psk[:], xtk_t[:, k, ts(s, 128)],
                                                 wtk_sb[:, k, :],
                                                 start=(k == 0), stop=False)
                            nc.tensor.matmul(psk[:], ones_row[:], bk_row[:],
                                             start=False, stop=True)
                            E2 = tmp.tile([128, 512], F32, tag="E")
                            nc.scalar.activation(E2[:], psk[:], ACTF.Exp)
                            R2 = tmp.tile([128, 512], F32, tag="R")
                            nc.scalar.activation(R2[:], psk[:], ACTF.Relu)
                            Ksb = tmp.tile([128, 512], BF16, tag="Ksb", bufs=5)
                            nc.vector.scalar_tensor_tensor(Ksb[:], E2[:], 1.0,
                                                           R2[:], ALU.min, ALU.add)
                            Ksbs.append(Ksb)

                        # ---- V projections + mask/scale, 4 subtiles
                        for s in range(4):
                            st = b * 4 + s
                            psv = psA.tile([128, 512], F32, tag="psv")
                            for k in range(KC):
                                nc.tensor.matmul(psv[:], xtv_t[:, k, ts(s, 128)],
                                                 wtv_sb[:, k, :],
                                                 start=(k == 0), stop=False)
                            nc.tensor.matmul(psv[:], ones_row[:], bv_row[:],
                                             start=False, stop=True)
                            V1 = tmp.tile([128, HPC, 65], BF16, tag="V1", bufs=5)
                            nc.vector.tensor_scalar(
                                V1[:, :, 0:64],
                                psv.rearrange("p (h d) -> p h d", d=64),
                                mm2L[:, st:st + 1], None, ALU.mult)
                            nc.vector.tensor_copy(out=V1[:, :, 64:65],
                                                  in_=mkv8[:, st, :, None])
                            V1s.append(V1)

                        # ---- Q^T tiles: (128 feat, 512 tok), weights stationary
                        for f in range(FT):
                            psq = psA.tile([128, 512], F32, tag="psq")
                            for k in range(KC):
                                nc.tensor.matmul(psq[:], wtq_sb[:, k, ts(f, 128)],
                                                 xtq_t[:, k, :],
                                                 start=(k == 0), stop=(k == KC - 1))
                            E = tmp.tile([128, 512], F32, tag="E")
                            nc.scalar.activation(E[:], psq[:], ACTF.Exp,
                                                 bias=bq_pp[:, f:f + 1])
                            R = tmp.tile([128, 512], F32, tag="R")
                            nc.scalar.activation(R[:], psq[:], ACTF.Relu,
                                                 bias=bq_pp[:, f:f + 1])
                            nc.vector.scalar_tensor_tensor(
                                QT[:, f, ds(b * 512, 512)], E[:], 1.0, R[:],
                                ALU.min, ALU.add)

                        # ---- kv accumulation, 4 subtiles x 8 heads
                        for s in range(4):
                            st = b * 4 + s
                            for h in range(HPC):
                                kvp = kvA if h % 2 == 0 else kvB
                                nc.tensor.matmul(
                                    kvp[:, h // 2, :],
                                    Ksbs[s][:, ts(h, 64)],
                                    V1s[s][:, h, :],
                                    start=(st == 0 and h < 2),
                                    stop=(st == NST - 1 and h // 2 == FT - 1),
                                    skip_group_check=True)

                # block-diagonal kv for the output einsum:
                # rows 0:64 = even head of pair (cols 0:65), rows 64:128 = odd
                kv_sb = singles.tile([128, FT, 130], BF16)
                nc.vector.memset(kv_sb[:], 0.0)
                nc.vector.tensor_copy(out=kv_sb[0:64, :, 0:65], in_=kvA[:])
                nc.vector.tensor_copy(out=kv_sb[64:128, :, 65:130], in_=kvB[:])

            with tc.tile_pool(name="psB", bufs=2, space="PSUM") as psB, \
                 nc.named_scope("phaseB"):
                for g in range(NST // 4):
                    poAs, poBs = [], []
                    qs32 = kvout.tile([128, 4, HPC], F32, tag="qs32", bufs=2)
                    for j in range(4):
                        st = g * 4 + j
                        poA = psB.tile([128, 2, 130], F32, tag="poA", bufs=4)
                        poB = psB.tile([128, 2, 130], F32, tag="poB", bufs=4)
                        for f in range(FT):
                            po = poA if f < 2 else poB
                            nc.tensor.matmul(po[:, f % 2, :],
                                             QT[:, f, ts(st, 128)],
                                             kv_sb[:, f, :],
                                             start=True, stop=True,
                                             skip_group_check=True)
                        nc.vector.tensor_copy(
                            out=qs32[:, j, 0:4].rearrange("p (f c) -> p f c", c=2),
                            in_=poA[:, :, 64:130:65])
                        nc.vector.tensor_copy(
                            out=qs32[:, j, 4:8].rearrange("p (f c) -> p f c", c=2),
                            in_=poB[:, :, 64:130:65])
                        poAs.append(poA)
                        poBs.append(poB)
                    # batched scale chain for 4 subtiles:
                    # s = m_q*L / (eps + m_q*qs)
                    mq_g = mq_sb[:, ds(g * 4, 4), None]
                    den32 = kvout.tile([128, 4, HPC], F32, tag="den32", bufs=2)
                    nc.vector.tensor_tensor(out=den32[:], in0=qs32[:],
                                            in1=mq_g.to_broadcast((128, 4, HPC)),
                                            op=ALU.mult)
                    nc.vector.tensor_scalar(den32[:], den32[:], EPS, None, ALU.add)
                    rcp32 = kvout.tile([128, 4, HPC], F32, tag="rcp32", bufs=2)
                    nc.vector.reciprocal(rcp32[:], den32[:])
                    s32 = kvout.tile([128, 4, HPC], F32, tag="s32", bufs=2)
                    nc.vector.tensor_tensor(out=s32[:], in0=rcp32[:],
                                            in1=mqL[:, ds(g * 4, 4), None]
                                            .to_broadcast((128, 4, HPC)),
                                            op=ALU.mult)
                    for j in range(4):
                        st = g * 4 + j
                        poA, poB = poAs[j], poBs[j]
                        outsb = kvout.tile([128, HPC, 64], F32, tag="outsb",
                                           bufs=4)
                        # heads 0-3 scaled on ScalarE (idle in phase B), 4-7 DVE
                        for h in range(4):
                            f, par = h // 2, h % 2
                            cols = slice(0, 64) if par == 0 else slice(65, 129)
                            nc.scalar.activation(outsb[:, h, :], poA[:, f, cols],
                                                 ACTF.Identity, bias=0.0,
                                                 scale=s32[:, j, h:h + 1])
                        nc.vector.tensor_tensor(
                            out=outsb[:, 4:8:2, :], in0=poB[:, :, 0:64],
                            in1=s32[:, j, 4:8:2].to_broadcast((128, 2, 64)),
                            op=ALU.mult)
                        nc.vector.tensor_tensor(
                            out=outsb[:, 5:8:2, :], in0=poB[:, :, 65:129],
                            in1=s32[:, j, 5:8:2].to_broadcast((128, 2, 64)),
                            op=ALU.mult)
                        nc.sync.dma_start(out_d.ap()[ds(st * 128, 128), :],
                                          outsb.rearrange("p h d -> p (h d)"))

    nc.compile()
    return nc


def _get_program():
    global _PROG
    if _PROG is None:
        _PROG = _build_program()
    return _PROG


def kernel(query, key, value, mask_q, mask_kv, Wq, bq, Wk, bk, Wv, bv):
    query = np.asarray(query, dtype=np.float32)
    key = np.asarray(key, dtype=np.float32)
    value = np.asarray(value, dtype=np.float32)
    mask_q = np.asarray(mask_q, dtype=np.float32)
    mask_kv = np.asarray(mask_kv, dtype=np.float32)
    Wq = np.asarray(Wq, dtype=np.float32)
    bq = np.asarray(bq, dtype=np.float32)
    Wk = np.asarray(Wk, dtype=np.float32)
    bk = np.asarray(bk, dtype=np.float32)
    Wv = np.asarray(Wv, dtype=np.float32)
    bv = np.asarray(bv, dtype=np.float32)

    nc = _get_program()

    xt = {}
    for b in range(B):
        xt[b] = (np.ascontiguousarray(query[b].T),
                 np.ascontiguousarray(key[b].T),
                 np.ascontiguousarray(value[b].T))
    wslices = {}
    for hh in range(2):
        sl = slice(hh * F, (hh + 1) * F)
        wslices[hh] = (
            np.ascontiguousarray(Wq[sl, :].T),
            np.ascontiguousarray(Wk[sl, :].T),
            np.ascontiguousarray(Wv[sl, :].T),
            bq[sl].copy(),
            bk[sl].astype(ml_dtypes.bfloat16).reshape(1, F),
            bv[sl].astype(ml_dtypes.bfloat16).reshape(1, F),
        )

    in_maps = []
    for core in range(NCORES):
        b, hh = core // 2, core % 2
        xtq, xtk, xtv = xt[b]
        wtq, wtk, wtv, bq_h, bk_h, bv_h = wslices[hh]
        in_maps.append({
            "xtq": xtq, "xtk": xtk, "xtv": xtv,
            "wtq": wtq, "wtk": wtk, "wtv": wtv,
            "bq": bq_h, "bk": bk_h, "bv": bv_h,
            "mq": mask_q[b], "mkv": mask_kv[b],
        })

    trace = os.environ.get("TRN_KERNEL_TRACE", "0") == "1"
    trace_cores = list(range(NCORES)) if trace else None
    res = run_bass_kernel_spmd(nc, in_maps, list(range(NCORES)),
                               trace=trace, trace_cores=trace_cores)
    if trace:
        kernel.last_exec_time_ns = res.exec_time_ns
        kernel.last_scope_times = res.per_core_scope_times

    out = np.empty((B, L, H, D), dtype=np.float32)
    for core in range(NCORES):
        b, hh = core // 2, core % 2
        out[b, :, hh * HPC:(hh + 1) * HPC, :] = \
            res.results[core]["out"].reshape(L, HPC, D)
    return out
